# revision 7
# baseline (speedup 1.0000x reference)
"""Trainium2 Bass kernel for ChebConv with spatial attention.

Reference computation (per sample b):
    A_k = cheb[k] * att[b]                    (elementwise, [N,N])
    rhs_k = A_k @ x[b,t]                      ([N,N] @ [N,F_IN] for all t)
    out[b,t] = relu(sum_k rhs_k @ Theta[k])   ([N,F_OUT])

Sharding: data-parallel over batch B=8, one sample per NeuronCore.
cheb/Theta replicated. Host prep passes transposed adjacency factors
(attT/chebT, layout [j,i], cast to bf16) so the on-chip elementwise
product directly yields A^T tiles, which the PE consumes as the moving
matmul operand with contraction over j on the partition dim — no
on-chip transposes. All matmuls run in bf16 (single-pass on the PE,
fp32 PSUM accumulation); the relu'd output is stored bf16 on-chip and
upcast to fp32 on the host.

Per-core dataflow:
  phase B: for k, j-tile: AT = attT*chebT (DVE bf16), then accumulate
           RT[(t,f)=128, i=512] = X_tile^T @ AT into PSUM over j-tiles
           (N=512 bf16 matmuls), copy+cast PSUM->SBUF bf16 (DVE + ACT).
  phase C: out[i=128, (t,o)] += RT^T @ thetaM_k, where thetaM zero-pads
           Theta[k] per 32-row strip so one full-K matmul produces the
           4 t's of a t-group (N=256). relu split ACT/DVE, DMA out.

Schedule notes (v2):
  - All input tiles are SBUF-resident with dedicated buffers; every
    input DMA is issued up-front in priority order with no WAR coupling
    to compute, so the two HWDGE queues free-run at HBM rate from the
    first possible cycle.  k=0 is DMA-bound (att+x+cheb0 = 5 MB against
    13.8 us of PE work), so its bytes are split evenly across the sync
    and scalar queues and ordered j-tile-first; cheb1 rides scalar and
    cheb2 sync strictly behind the k=0-critical transfers.
  - ~3 us of zero-operand warm-up matmuls run during the DMA head so
    the PE HAM clock gate releases (1.2 -> 2.4 GHz) before real work.
  - One PSUM pool serves both phases: phase C's accumulators reuse the
    chain banks in the order k=2's PSUM->SBUF copies retire (psA on
    c0/c2, psB on c4/c6, tg in copy order), hiding the B->C turnover.
  - relu+cast per i-block is split psA->ACT, psB->DVE so neither engine
    throttles phase C's matmul cadence; the last i-block's output DMA
    is issued in halves to shorten the tail.
"""

import numpy as np
from contextlib import ExitStack

B, T, N, F_IN, F_OUT, K = 8, 16, 1024, 32, 64, 3
NJ = N // 128   # j tiles (contraction)
NI = N // 128   # i tiles (output rows)
NTG = 4         # t-groups of 4 t's -> 128 = 4*32 partitions
TF = T * F_IN   # 512
TO = T * F_OUT  # 1024
W = 2           # j-tiles per wide DMA

_LAST_RESULTS = None  # BassKernelResults of the most recent run (for test harness)

# cheb j-tile grouping per k: k=0 is fed j-tile-at-a-time (DMA-critical
# window); later k's use wide pairs only.
CHEB_GROUPS = {
    0: [(j,) for j in range(8)],
    1: [(0, 1), (2, 3), (4, 5), (6, 7)],
    2: [(0, 1), (2, 3), (4, 5), (6, 7)],
}


def _build_bass():
    import concourse.mybir as mybir
    import concourse.tile as tile
    from concourse import bacc
    from concourse.bass import ts

    f32 = mybir.dt.float32
    bf16 = mybir.dt.bfloat16
    nc = bacc.Bacc()

    xT_d = nc.dram_tensor("xT", [N, TF], bf16, kind="ExternalInput")
    attT_d = nc.dram_tensor("attT", [N, N], bf16, kind="ExternalInput")
    chebT_d = nc.dram_tensor("chebT", [K * N, N], bf16, kind="ExternalInput")
    th_d = nc.dram_tensor("thetaM", [128, K * 4 * F_OUT], bf16, kind="ExternalInput")
    out_d = nc.dram_tensor("out", [N, TO], bf16, kind="ExternalOutput")

    def grouped(dram, row0, L):  # L j-tiles -> [128, L, cols]
        return dram[row0:row0 + L * 128, :].rearrange("(a p) n -> p a n", p=128)

    def g3(t, L):  # view a grouped SBUF tile as [128, L, cols]
        return t[:].rearrange("p (a n) -> p a n", a=L)

    with tile.TileContext(nc) as tc, ExitStack() as ctx:
        x_pool = ctx.enter_context(tc.tile_pool(name="x", bufs=1))
        att_pool = ctx.enter_context(tc.tile_pool(name="att", bufs=1))
        cheb_pool = ctx.enter_context(tc.tile_pool(name="cheb", bufs=1))
        at_pool = ctx.enter_context(tc.tile_pool(name="at", bufs=3))
        rt_pool = ctx.enter_context(tc.tile_pool(name="rt", bufs=1))
        th_pool = ctx.enter_context(tc.tile_pool(name="th", bufs=1))
        ob_pool = ctx.enter_context(tc.tile_pool(name="ob", bufs=3))
        warm_pool = ctx.enter_context(tc.tile_pool(name="warm", bufs=1))
        pp = ctx.enter_context(tc.tile_pool(name="psum", bufs=1, space="PSUM"))

        attg = [att_pool.tile([128, N], bf16, name=f"attg{j}", tag=f"attg{j}")
                for j in range(NJ)]
        xg = [x_pool.tile([128, TF], bf16, name=f"xg{j}", tag=f"xg{j}")
              for j in range(NJ)]
        cbt = {}
        for k in range(K):
            for g, grp in enumerate(CHEB_GROUPS[k]):
                L = len(grp)
                cbt[(k, g)] = cheb_pool.tile(
                    [128, L * N], bf16, name=f"cb{k}_{g}", tag=f"cb{k}_{g}")
        th = th_pool.tile([128, K * 4 * F_OUT], bf16)
        wz = warm_pool.tile([128, 512], bf16, name="warmz", tag="warmz")

        # ---- DMA program: call order == scheduler priority == per-queue
        # issue order.  Each HWDGE queue sustains only ~half of the
        # ~385 GB/s HBM rate, so the k=0-critical tensors (att, x, cheb0)
        # are interleaved across BOTH queues j-bundle by j-bundle: each
        # bundle's ~0.6 MB lands split evenly, keeping bundle-ready times
        # ~1.6 us apart, just ahead of the PE's 1.73 us/j-tile pace. ----
        def att_dma(j, eng):
            eng.dma_start(attg[j][:], attT_d[ts(j, 128), :])

        def x_dma(j, eng):
            eng.dma_start(xg[j][:], xT_d[ts(j, 128), :])

        def cb_dma(k, g, eng):
            grp = CHEB_GROUPS[k][g]
            eng.dma_start(g3(cbt[(k, g)], len(grp)),
                          grouped(chebT_d, k * N + grp[0] * 128, len(grp)))

        nc.gpsimd.memset(wz[:], 0)
        for j in range(NJ):
            # round j: sync and scalar each carry half the bundle,
            # alternating which queue gets cheb vs att
            if j % 2 == 0:
                att_dma(j, nc.scalar)
                cb_dma(0, j, nc.sync)
                x_dma(j, nc.sync)
            else:
                att_dma(j, nc.sync)
                cb_dma(0, j, nc.scalar)
                x_dma(j, nc.scalar)
        nc.scalar.dma_start(th[:], th_d[:, :])
        cb_dma(1, 0, nc.sync)
        cb_dma(1, 2, nc.scalar)
        cb_dma(1, 1, nc.sync)
        cb_dma(1, 3, nc.scalar)
        cb_dma(2, 0, nc.sync)
        cb_dma(2, 2, nc.scalar)
        cb_dma(2, 1, nc.sync)
        cb_dma(2, 3, nc.scalar)

        # ---- PE warm-up: ~3us of zero matmuls during the DMA head so HAM
        # un-throttles before the first real matmul.  Uses chain c7's bank;
        # released before phase B's 8th matmul reaches it. ----
        wps = pp.tile([128, 512], f32, name="warmps", tag="c7")
        for _ in range(7):
            nc.tensor.matmul(wps[:], wz[:, 0:128], wz[:], start=True, stop=True)

        # ---- phase B: RT[k][tg] = X[:, tg-block]^T @ (attT * chebT_k) ----
        rts = [[None] * NTG for _ in range(K)]
        for k in range(K):
            chains = [pp.tile([128, 512], f32, name=f"ch{k}_{c}", tag=f"c{c}")
                      for c in range(2 * NTG)]
            n_j = 0
            for g, grp in enumerate(CHEB_GROUPS[k]):
                at = at_pool.tile([128, len(grp) * N], bf16, name=f"at{k}_{g}",
                                  tag="at", padded_shape=[128, W * N])
                cb = cbt[(k, g)]
                for js, j in enumerate(grp):
                    if k == 0 and g <= 1:
                        # split the first products so the first matmul only
                        # waits for half a tile
                        nc.vector.tensor_mul(at[:, ts(2 * js, 512)],
                                             attg[j][:, 0:512],
                                             cb[:, ts(2 * js, 512)])
                        nc.vector.tensor_mul(at[:, ts(2 * js + 1, 512)],
                                             attg[j][:, 512:1024],
                                             cb[:, ts(2 * js + 1, 512)])
                    else:
                        nc.vector.tensor_mul(at[:, ts(js, N)], attg[j][:],
                                             cb[:, ts(js, N)])
                for js, j in enumerate(grp):
                    for tg in range(NTG):
                        for ih in range(2):
                            nc.tensor.matmul(
                                chains[tg * 2 + ih][:],
                                xg[j][:, ts(tg, 128)],
                                at[:, ts(js * 2 + ih, 512)],
                                start=(n_j == 0),
                                stop=(n_j == NJ - 1),
                            )
                    n_j += 1
            for tg in range(NTG):
                rt = rt_pool.tile([128, N], bf16, name=f"rt{k}_{tg}",
                                  tag=f"rt{k}_{tg}")
                nc.vector.tensor_copy(rt[:, 0:512], chains[tg * 2][:])
                nc.scalar.copy(rt[:, 512:1024], chains[tg * 2 + 1][:])
                rts[k][tg] = rt

        # ---- phase C: out[i-block, (t,o)] = relu(sum_k RT_k^T @ thetaM_k).
        # psA/psB reuse chain banks in k=2 copy-retirement order; tg order
        # matches so the B->C turnover overlaps the copies. ----
        for ib in range(NI):
            psA = pp.tile([128, 512], f32, name=f"psA{ib}",
                          tag=f"c{0 if ib % 2 == 0 else 2}")
            psB = pp.tile([128, 512], f32, name=f"psB{ib}",
                          tag=f"c{4 if ib % 2 == 0 else 6}")
            for tg, ps in ((0, psA), (1, psA), (2, psB), (3, psB)):
                for k in range(K):
                    nc.tensor.matmul(
                        ps[:, ts(tg % 2, 4 * F_OUT)],
                        rts[k][tg][:, ts(ib, 128)],
                        th[:, ts(k, 4 * F_OUT)],
                        start=(k == 0),
                        stop=(k == K - 1),
                    )
            ob = ob_pool.tile([128, TO], bf16, name=f"ob{ib}", tag="ob")
            nc.scalar.activation(ob[:, 0:512], psA[:],
                                 mybir.ActivationFunctionType.Relu)
            nc.vector.tensor_scalar_max(ob[:, 512:1024], psB[:], 0.0)
            if ib < NI - 1:
                nc.sync.dma_start(out_d[ts(ib, 128), :], ob[:])
            else:
                # halves so the tail transfer starts as soon as each relu lands
                nc.sync.dma_start(out_d[ts(ib, 128), 0:512], ob[:, 0:512])
                nc.sync.dma_start(out_d[ts(ib, 128), 512:1024], ob[:, 512:1024])

    nc.compile()
    return nc


def _prep_inputs(x, att, cheb, Theta):
    import ml_dtypes

    bf16 = ml_dtypes.bfloat16
    chebT = np.ascontiguousarray(cheb.transpose(0, 2, 1)).reshape(K * N, N)
    chebT = chebT.astype(bf16)
    # zero-padded Theta: strip tt of the partition dim carries Theta[k]
    # only in the tt-th 64-col block of k's 256-col group
    thetaM = np.zeros((128, K * 4 * F_OUT), np.float32)
    for tt in range(4):
        for k in range(K):
            thetaM[tt * 32:(tt + 1) * 32,
                   k * 4 * F_OUT + tt * F_OUT:
                   k * 4 * F_OUT + (tt + 1) * F_OUT] = Theta[k]
    thetaM = thetaM.astype(bf16)

    in_maps = []
    for b in range(B):
        in_maps.append({
            "xT": np.ascontiguousarray(
                x[b].transpose(1, 0, 2)).reshape(N, TF).astype(bf16),
            "attT": np.ascontiguousarray(att[b].T).astype(bf16),
            "chebT": chebT,
            "thetaM": thetaM,
        })
    return in_maps


def kernel(**inputs: np.ndarray) -> np.ndarray:
    global _LAST_RESULTS
    from concourse.bass_utils import run_bass_kernel_spmd

    x = np.asarray(inputs["x"], dtype=np.float32)
    att = np.asarray(inputs["spatial_attention"], dtype=np.float32)
    cheb = np.asarray(inputs["cheb"], dtype=np.float32)
    Theta = np.asarray(inputs["Theta"], dtype=np.float32)

    in_maps = _prep_inputs(x, att, cheb, Theta)
    nc = _build_bass()
    res = run_bass_kernel_spmd(nc, in_maps, core_ids=list(range(B)))
    _LAST_RESULTS = res

    out = np.stack(
        [r["out"].astype(np.float32).reshape(N, T, F_OUT).transpose(1, 0, 2)
         for r in res.results]
    )
    return out


# revision 12
# speedup vs baseline: 1.0278x; 1.0278x over previous
"""Trainium2 Bass kernel for ChebConv with spatial attention.

Reference computation (per sample b):
    A_k = cheb[k] * att[b]                    (elementwise, [N,N])
    rhs_k = A_k @ x[b,t]                      ([N,N] @ [N,F_IN] for all t)
    out[b,t] = relu(sum_k rhs_k @ Theta[k])   ([N,F_OUT])

Sharding: data-parallel over batch B=8, one sample per NeuronCore.
cheb/Theta replicated. Host prep passes transposed adjacency factors
(attT/chebT, layout [j,i], cast to bf16) so the on-chip elementwise
product directly yields A^T tiles, which the PE consumes as the moving
matmul operand with contraction over j on the partition dim — no
on-chip transposes. All matmuls run in bf16 (single-pass on the PE,
fp32 PSUM accumulation); the relu'd output is stored bf16 on-chip and
upcast to fp32 on the host.

Per-core dataflow:
  phase B: for k, j-tile: AT = attT*chebT (DVE bf16), then accumulate
           RT[(t,f)=128, i=512] = X_tile^T @ AT into PSUM over j-tiles
           (N=512 bf16 matmuls), copy+cast PSUM->SBUF bf16 (DVE + ACT).
  phase C: out[i=128, (t,o)] += RT^T @ thetaM_k, where thetaM zero-pads
           Theta[k] per 32-row strip so one full-K matmul produces the
           4 t's of a t-group (N=256). relu split ACT/DVE, DMA out.

Schedule notes (v2):
  - All input tiles are SBUF-resident with dedicated buffers; every
    input DMA is issued up-front in priority order with no WAR coupling
    to compute, so the two HWDGE queues free-run at HBM rate from the
    first possible cycle.  k=0 is DMA-bound (att+x+cheb0 = 5 MB against
    13.8 us of PE work), so its bytes are split evenly across the sync
    and scalar queues and ordered j-tile-first; cheb1 rides scalar and
    cheb2 sync strictly behind the k=0-critical transfers.
  - ~3 us of zero-operand warm-up matmuls run during the DMA head so
    the PE HAM clock gate releases (1.2 -> 2.4 GHz) before real work.
  - One PSUM pool serves both phases: phase C's accumulators reuse the
    chain banks in the order k=2's PSUM->SBUF copies retire (psA on
    c0/c2, psB on c4/c6, tg in copy order), hiding the B->C turnover.
  - relu+cast per i-block is split psA->ACT, psB->DVE so neither engine
    throttles phase C's matmul cadence; the last i-block's output DMA
    is issued in halves to shorten the tail.
"""

import numpy as np
from contextlib import ExitStack

B, T, N, F_IN, F_OUT, K = 8, 16, 1024, 32, 64, 3
NJ = N // 128   # j tiles (contraction)
NI = N // 128   # i tiles (output rows)
NTG = 4         # t-groups of 4 t's -> 128 = 4*32 partitions
TF = T * F_IN   # 512
TO = T * F_OUT  # 1024
W = 2           # j-tiles per wide DMA

_LAST_RESULTS = None  # BassKernelResults of the most recent run (for test harness)

# j-tile grouping: k=0 (and att/x) lead with two single tiles so the
# first matmuls start early; everything else uses wide pairs, which
# sustain materially higher per-queue DMA throughput than singles.
G5 = [(0,), (1,), (2, 3), (4, 5), (6, 7)]
CHEB_GROUPS = {
    0: G5,
    1: [(0, 1), (2, 3), (4, 5), (6, 7)],
    2: [(0, 1), (2, 3), (4, 5), (6, 7)],
}


def _build_bass():
    import concourse.mybir as mybir
    import concourse.tile as tile
    from concourse import bacc
    from concourse.bass import ts

    f32 = mybir.dt.float32
    bf16 = mybir.dt.bfloat16
    nc = bacc.Bacc()

    xT_d = nc.dram_tensor("xT", [N, TF], bf16, kind="ExternalInput")
    attT_d = nc.dram_tensor("attT", [N, N], bf16, kind="ExternalInput")
    chebT_d = nc.dram_tensor("chebT", [K * N, N], bf16, kind="ExternalInput")
    th_d = nc.dram_tensor("thetaM", [128, K * 4 * F_OUT], bf16, kind="ExternalInput")
    out_d = nc.dram_tensor("out", [N, TO], bf16, kind="ExternalOutput")

    def grouped(dram, row0, L):  # L j-tiles -> [128, L, cols]
        return dram[row0:row0 + L * 128, :].rearrange("(a p) n -> p a n", p=128)

    def g3(t, L):  # view a grouped SBUF tile as [128, L, cols]
        return t[:].rearrange("p (a n) -> p a n", a=L)

    with tile.TileContext(nc) as tc, ExitStack() as ctx:
        x_pool = ctx.enter_context(tc.tile_pool(name="x", bufs=1))
        att_pool = ctx.enter_context(tc.tile_pool(name="att", bufs=1))
        cheb_pool = ctx.enter_context(tc.tile_pool(name="cheb", bufs=1))
        at_pool = ctx.enter_context(tc.tile_pool(name="at", bufs=3))
        rt_pool = ctx.enter_context(tc.tile_pool(name="rt", bufs=1))
        th_pool = ctx.enter_context(tc.tile_pool(name="th", bufs=1))
        ob_pool = ctx.enter_context(tc.tile_pool(name="ob", bufs=3))
        warm_pool = ctx.enter_context(tc.tile_pool(name="warm", bufs=1))
        pp = ctx.enter_context(tc.tile_pool(name="psum", bufs=1, space="PSUM"))

        # att/x tiles follow the G5 grouping; per-j views via att_ap/x_ap
        attt = [att_pool.tile([128, len(grp) * N], bf16, name=f"attg{g}",
                              tag=f"attg{g}") for g, grp in enumerate(G5)]
        xt = [x_pool.tile([128, len(grp) * TF], bf16, name=f"xg{g}",
                          tag=f"xg{g}") for g, grp in enumerate(G5)]
        JG = {j: (g, grp.index(j)) for g, grp in enumerate(G5) for j in grp}

        def att_ap(j, c0, c1):
            g, a = JG[j]
            return attt[g][:, a * N + c0:a * N + c1]

        def x_chunk(j, tg):
            g, a = JG[j]
            return xt[g][:, a * TF + tg * 128:a * TF + (tg + 1) * 128]
        cbt = {}
        for k in range(K):
            for g, grp in enumerate(CHEB_GROUPS[k]):
                L = len(grp)
                cbt[(k, g)] = cheb_pool.tile(
                    [128, L * N], bf16, name=f"cb{k}_{g}", tag=f"cb{k}_{g}")
        th = th_pool.tile([128, K * 4 * F_OUT], bf16)
        wz = warm_pool.tile([128, 512], bf16, name="warmz", tag="warmz")

        # ---- DMA program: call order == scheduler priority == per-queue
        # issue order.  Each HWDGE queue sustains only ~half the ~390 GB/s
        # HBM rate, and small transfers degrade per-queue throughput
        # further, so the k=0-critical bytes (att + x + cheb0 = 5 MB
        # against 13.8 us of k=0 PE work) are split evenly between the
        # sync and scalar queues as wide transfers in j-need order, while
        # x2-x7 and thetaM ride the gpsimd software-DGE queue as a third
        # path.  cheb1/cheb2 queue strictly behind the k=0 bytes. ----
        def att_dma(g, eng):
            grp = G5[g]
            eng.dma_start(g3(attt[g], len(grp)),
                          grouped(attT_d, grp[0] * 128, len(grp)))

        def x_dma(g, eng):
            grp = G5[g]
            eng.dma_start(g3(xt[g], len(grp)),
                          grouped(xT_d, grp[0] * 128, len(grp)))

        def cb_dma(k, g, eng):
            grp = CHEB_GROUPS[k][g]
            eng.dma_start(g3(cbt[(k, g)], len(grp)),
                          grouped(chebT_d, k * N + grp[0] * 128, len(grp)))

        nc.gpsimd.memset(wz[:], 0)
        att_dma(0, nc.scalar)
        cb_dma(0, 0, nc.sync)
        x_dma(0, nc.sync)
        att_dma(1, nc.scalar)
        cb_dma(0, 1, nc.sync)
        x_dma(1, nc.scalar)
        x_dma(2, nc.gpsimd)
        att_dma(2, nc.scalar)
        cb_dma(0, 2, nc.sync)
        x_dma(3, nc.gpsimd)
        att_dma(3, nc.scalar)
        cb_dma(0, 3, nc.sync)
        x_dma(4, nc.gpsimd)
        att_dma(4, nc.scalar)
        cb_dma(0, 4, nc.sync)
        nc.gpsimd.dma_start(th[:], th_d[:, :])
        cb_dma(1, 0, nc.sync)
        cb_dma(1, 2, nc.scalar)
        cb_dma(1, 1, nc.sync)
        cb_dma(1, 3, nc.scalar)
        cb_dma(2, 0, nc.sync)
        cb_dma(2, 2, nc.scalar)
        cb_dma(2, 1, nc.sync)
        cb_dma(2, 3, nc.scalar)

        # ---- PE warm-up: ~3us of zero matmuls during the DMA head so HAM
        # un-throttles before the first real matmul.  Uses chain c7's bank;
        # released before phase B's 8th matmul reaches it. ----
        wps = pp.tile([128, 512], f32, name="warmps", tag="c7")
        for _ in range(7):
            nc.tensor.matmul(wps[:], wz[:, 0:128], wz[:], start=True, stop=True)

        # ---- phase B: RT[k][tg] = X[:, tg-block]^T @ (attT * chebT_k) ----
        rts = [[None] * NTG for _ in range(K)]
        for k in range(K):
            chains = [pp.tile([128, 512], f32, name=f"ch{k}_{c}", tag=f"c{c}")
                      for c in range(2 * NTG)]
            n_j = 0
            for g, grp in enumerate(CHEB_GROUPS[k]):
                at = at_pool.tile([128, len(grp) * N], bf16, name=f"at{k}_{g}",
                                  tag="at", padded_shape=[128, W * N])
                cb = cbt[(k, g)]
                for js, j in enumerate(grp):
                    if k == 0 and g <= 1:
                        # split the first products so the first matmul only
                        # waits for half a tile
                        nc.vector.tensor_mul(at[:, ts(2 * js, 512)],
                                             att_ap(j, 0, 512),
                                             cb[:, ts(2 * js, 512)])
                        nc.vector.tensor_mul(at[:, ts(2 * js + 1, 512)],
                                             att_ap(j, 512, 1024),
                                             cb[:, ts(2 * js + 1, 512)])
                    else:
                        nc.vector.tensor_mul(at[:, ts(js, N)],
                                             att_ap(j, 0, N),
                                             cb[:, ts(js, N)])
                for js, j in enumerate(grp):
                    for tg in range(NTG):
                        for ih in range(2):
                            nc.tensor.matmul(
                                chains[tg * 2 + ih][:],
                                x_chunk(j, tg),
                                at[:, ts(js * 2 + ih, 512)],
                                start=(n_j == 0),
                                stop=(n_j == NJ - 1),
                            )
                    n_j += 1
            for tg in range(NTG):
                rt = rt_pool.tile([128, N], bf16, name=f"rt{k}_{tg}",
                                  tag=f"rt{k}_{tg}")
                nc.vector.tensor_copy(rt[:, 0:512], chains[tg * 2][:])
                nc.scalar.copy(rt[:, 512:1024], chains[tg * 2 + 1][:])
                rts[k][tg] = rt

        # ---- phase C: out[i-block, (t,o)] = relu(sum_k RT_k^T @ thetaM_k).
        # psA/psB reuse chain banks in k=2 copy-retirement order; tg order
        # matches so the B->C turnover overlaps the copies. ----
        for ib in range(NI):
            psA = pp.tile([128, 512], f32, name=f"psA{ib}",
                          tag=f"c{0 if ib % 2 == 0 else 2}")
            psB = pp.tile([128, 512], f32, name=f"psB{ib}",
                          tag=f"c{4 if ib % 2 == 0 else 6}")
            for tg, ps in ((0, psA), (1, psA), (2, psB), (3, psB)):
                for k in range(K):
                    nc.tensor.matmul(
                        ps[:, ts(tg % 2, 4 * F_OUT)],
                        rts[k][tg][:, ts(ib, 128)],
                        th[:, ts(k, 4 * F_OUT)],
                        start=(k == 0),
                        stop=(k == K - 1),
                    )
            ob = ob_pool.tile([128, TO], bf16, name=f"ob{ib}", tag="ob")
            nc.scalar.activation(ob[:, 0:512], psA[:],
                                 mybir.ActivationFunctionType.Relu)
            nc.vector.tensor_scalar_max(ob[:, 512:1024], psB[:], 0.0)
            if ib < NI - 1:
                nc.sync.dma_start(out_d[ts(ib, 128), :], ob[:])
            else:
                # halves so the tail transfer starts as soon as each relu lands
                nc.sync.dma_start(out_d[ts(ib, 128), 0:512], ob[:, 0:512])
                nc.sync.dma_start(out_d[ts(ib, 128), 512:1024], ob[:, 512:1024])

    nc.compile()
    return nc


def _prep_inputs(x, att, cheb, Theta):
    import ml_dtypes

    bf16 = ml_dtypes.bfloat16
    chebT = np.ascontiguousarray(cheb.transpose(0, 2, 1)).reshape(K * N, N)
    chebT = chebT.astype(bf16)
    # zero-padded Theta: strip tt of the partition dim carries Theta[k]
    # only in the tt-th 64-col block of k's 256-col group
    thetaM = np.zeros((128, K * 4 * F_OUT), np.float32)
    for tt in range(4):
        for k in range(K):
            thetaM[tt * 32:(tt + 1) * 32,
                   k * 4 * F_OUT + tt * F_OUT:
                   k * 4 * F_OUT + (tt + 1) * F_OUT] = Theta[k]
    thetaM = thetaM.astype(bf16)

    in_maps = []
    for b in range(B):
        in_maps.append({
            "xT": np.ascontiguousarray(
                x[b].transpose(1, 0, 2)).reshape(N, TF).astype(bf16),
            "attT": np.ascontiguousarray(att[b].T).astype(bf16),
            "chebT": chebT,
            "thetaM": thetaM,
        })
    return in_maps


def kernel(**inputs: np.ndarray) -> np.ndarray:
    global _LAST_RESULTS
    from concourse.bass_utils import run_bass_kernel_spmd

    x = np.asarray(inputs["x"], dtype=np.float32)
    att = np.asarray(inputs["spatial_attention"], dtype=np.float32)
    cheb = np.asarray(inputs["cheb"], dtype=np.float32)
    Theta = np.asarray(inputs["Theta"], dtype=np.float32)

    in_maps = _prep_inputs(x, att, cheb, Theta)
    nc = _build_bass()
    res = run_bass_kernel_spmd(nc, in_maps, core_ids=list(range(B)))
    _LAST_RESULTS = res

    out = np.stack(
        [r["out"].astype(np.float32).reshape(N, T, F_OUT).transpose(1, 0, 2)
         for r in res.results]
    )
    return out
